# revision 33
# baseline (speedup 1.0000x reference)
"""Trainium2 Bass kernel for nn_BodyKinematics (batched tree forward kinematics).

Contract: kernel(**inputs) takes FULL unsharded numpy inputs, returns the FULL
output (B, N, 4, 4) float32.  Batch dim sharded across 8 NeuronCores.

Device math (fp16 storage, DVE fp32-internal; rel-err ~4e-3 vs 2e-2 budget):
  theta = tanh(la) * sc                                  # ACT, fp32
  sin = Sin(sc*t), cos = Sin(pi/2 - sc*|t|)              # ACT -> fp16
  local_e = Rx@Ry@Rz @ tip_e  (Givens chain, 18 TT ops)  # DVE fp16 2x
  tree:  W_n = W_parent @ local  (levelwise)             # DVE fp16 2x
  out = W rows 0:3 as fp16 "planar" planes; host casts to fp32, permutes
  sigma->BFS, and adds the constant bottom rows.

Layout: "sigma-planar".  Per (row i, col l) of the 3x4 transforms there is a
plane of 256 slots (one per tree node).  Slots are level-blocks [2^(L-1), 2^L)
with children of the slot-ordered parents laid left-block then right-block, so
every tree op reads/writes stride-1 runs (fp16 2x_1p eligible).  Slot 0 holds
BFS node 255 (the lone level-9 child), computed early via a local-pair product
so the slot [0,64) output DMA can fire before level 8.  The host pre-permutes
log_angles columns (axis-major, slot order) and the tip planes to match.
"""

import os
import sys

for _p in ("/opt/trn_rl_repo",):
    if _p not in sys.path and os.path.isdir(_p):
        sys.path.insert(0, _p)

import numpy as np

B, E, N = 4096, 255, 256
J = 3 * E
NCORE, P, NSUB = 8, 128, 4
BPC = P * NSUB
PI = float(np.pi)
JP = 768            # padded per-sub angle columns (3 * 256)
OUTW = 3072         # 12 planes * 256 slots

_state: dict = {}

# experiment flags
POOL_BC_SUB = None      # int sub index whose BC runs on Pool (None = all DVE)
                        # NOTE: Pool TT on fp16 crashes the exec unit — keep None
POOL_BC_F32 = False     # sub-3 BC on Pool in fp32 scratch + ACT cast to fp16
                        # (measured 148us vs 124us all-DVE — Pool TT is far
                        # below its cost-model efficiency on these patterns)
POOL_TREE_LVLS = ()     # levels whose sub-POOL_BC_SUB muls run on Pool


# --------------------------------------------------------------------------- #
# slot maps (sigma ordering)
# --------------------------------------------------------------------------- #
def _slot_maps():
    node_of_slot = np.full(256, -1, np.int64)
    node_of_slot[1] = 0
    for lvl in range(2, 9):
        blo = 1 << (lvl - 1)
        q = blo >> 1
        pblo = blo >> 1
        for j in range(q):
            par = node_of_slot[pblo + j]
            node_of_slot[blo + j] = 2 * par + 1
            node_of_slot[blo + q + j] = 2 * par + 2
    node_of_slot[0] = 255
    return node_of_slot


NODE_OF_SLOT = _slot_maps()
# edge feeding each slot (slot 1 = root has no edge)
EDGE_SLOTS = np.array([s for s in range(256) if s != 1], np.int64)
EDGE_OF_SLOT = NODE_OF_SLOT[EDGE_SLOTS] - 1  # (255,)


# --------------------------------------------------------------------------- #
# numpy fallback (exact float32 port of the reference)
# --------------------------------------------------------------------------- #
def _np_skew(a):
    x, y, z = a[..., 0], a[..., 1], a[..., 2]
    zero = np.zeros_like(x)
    return np.stack([
        np.stack([zero, -z, y], -1),
        np.stack([z, zero, -x], -1),
        np.stack([-y, x, zero], -1)], -2)


def _np_fallback(log_angles, tip_to_base, rot_axes, rot_constraints):
    la = log_angles.astype(np.float32)
    b, e3 = la.shape
    e = e3 // 3
    n = e + 1
    theta = np.tanh(la) * rot_constraints[:, 0] + rot_constraints[:, 1]
    K = _np_skew(rot_axes.astype(np.float32))
    K2 = np.einsum('mij,mjk->mik', K, K).astype(np.float32)
    s = np.sin(theta)[..., None, None]
    c = (1.0 - np.cos(theta))[..., None, None]
    I3 = np.eye(3, dtype=np.float32)
    rots = (I3 + s * K + c * K2).reshape(b, e, 3, 3, 3).astype(np.float32)
    r = np.einsum('beij,bejk,bekl->beil', rots[:, :, 0], rots[:, :, 1],
                  rots[:, :, 2]).astype(np.float32)
    T = np.zeros((b, e, 4, 4), np.float32)
    T[..., :3, :3] = r
    T[..., 3, 3] = 1.0
    local = np.einsum('beij,ejk->beik', T,
                      tip_to_base.astype(np.float32)).astype(np.float32)
    worlds = np.zeros((b, n, 4, 4), np.float32)
    worlds[:, 0] = np.eye(4, dtype=np.float32)
    for i in range(1, n):
        par = (i - 1) // 2
        worlds[:, i] = (worlds[:, par] @ local[:, i - 1]).astype(np.float32)
    return worlds


# --------------------------------------------------------------------------- #
# host-side feed construction / output assembly
# --------------------------------------------------------------------------- #
def make_feeds(log_angles, tip_to_base):
    la3 = np.asarray(log_angles, np.float32).reshape(-1, E, 3)
    bb = la3.shape[0]
    la_feed = np.zeros((bb, 3, 256), np.float32)
    la_feed[:, :, EDGE_SLOTS] = la3[:, EDGE_OF_SLOT, :].transpose(0, 2, 1)
    la_feed = la_feed.reshape(bb, JP)

    tip_feed = np.zeros((3, 4, 256), np.float16)
    tip_feed[:, :, EDGE_SLOTS] = (
        np.asarray(tip_to_base, np.float32)[EDGE_OF_SLOT, :3, :]
        .astype(np.float16).transpose(1, 2, 0))
    tip_feed = tip_feed.reshape(1, OUTW)
    return la_feed, tip_feed


def assemble_output(dev_out):
    # dev_out: (B, 3072) fp16, sigma-planar -> (B, N, 4, 4) fp32 BFS
    v = np.asarray(dev_out).reshape(B, 3, 4, 256).astype(np.float32)
    out4 = np.zeros((B, N, 4, 4), np.float32)
    out4[:, NODE_OF_SLOT, :3, :] = v.transpose(0, 3, 1, 2)
    out4[:, :, 3, 3] = 1.0
    return out4


# --------------------------------------------------------------------------- #
# device kernel build
# --------------------------------------------------------------------------- #
def _build_nc(sc_const: float, loop_n: int = 1):
    import concourse.bacc as bacc
    import concourse.mybir as mybir
    from concourse.tile import TileContext
    import concourse.bass as bass
    from contextlib import ExitStack

    f32 = mybir.dt.float32
    f16 = mybir.dt.float16
    Alu = mybir.AluOpType
    AFT = mybir.ActivationFunctionType
    sc = float(sc_const)

    nc = bacc.Bacc("TRN2", target_bir_lowering=False, debug=False)

    la_d = nc.dram_tensor("la", [BPC, JP], f32, kind="ExternalInput")
    tip_d = nc.dram_tensor("tip", [1, OUTW], f16, kind="ExternalInput")
    out_d = nc.dram_tensor("out", [BPC, OUTW], f16, kind="ExternalOutput")

    with TileContext(nc) as tc:
        with tc.tile_pool(name="main", bufs=1) as pool, ExitStack() as _lc:
            if loop_n > 1:
                _lc.enter_context(tc.For_i(0, loop_n, 1))

            nbc = NSUB - 1 if POOL_BC_F32 else NSUB
            la_t = pool.tile([P, NSUB * JP], f32)
            th_t = pool.tile([P, NSUB * JP], f32)
            tg_t = pool.tile([P, NSUB * 1536], f16)
            tip_t = pool.tile([P, OUTW], f16)
            tA_t = pool.tile([P, nbc * 1024], f16)
            r0_t = pool.tile([P, nbc * 1024], f16)
            r1_t = pool.tile([P, nbc * 1024], f16)
            q2_t = pool.tile([P, nbc * 1024], f16)
            loc_t = pool.tile([P, NSUB * OUTW], f16)
            w_t = pool.tile([P, NSUB * OUTW], f16)
            tmp_t = pool.tile([P, NSUB * OUTW], f16)
            if POOL_BC_F32:
                tip32_t = pool.tile([P, OUTW], f32)
                tg32_t = pool.tile([P, 1536], f32)
                tA32_t = pool.tile([P, 1024], f32)
                r032_t = pool.tile([P, 1024], f32)
                r132_t = pool.tile([P, 1024], f32)
                q232_t = pool.tile([P, 1024], f32)
                loc32_t = pool.tile([P, OUTW], f32)

            def ap(tile, off, dims):
                a = tile[:]
                return bass.AP(a.tensor, a.offset + off,
                               [list(a.ap[0])] + [list(d) for d in dims])

            tt = nc.vector.tensor_tensor
            act = nc.scalar.activation

            # ---------------- input DMAs ----------------
            la_v = la_d[:].rearrange("(s p) j -> p s j", p=P)
            for s in range(NSUB):
                nc.sync.dma_start(la_t[:, s * JP:(s + 1) * JP], la_v[:, s])
            for c in range(4):
                nc.sync.dma_start(
                    tip_t[:, c * JP:(c + 1) * JP],
                    bass.AP(tip_d, c * JP, [[0, P], [1, JP]]))

            # ---------------- trig (ACT) ----------------
            hpi_t = pool.tile([P, 1], f32)
            nc.gpsimd.memset(hpi_t[:], PI / 2.0)
            if POOL_BC_F32:
                # fp32 tip copy for the Pool BC island
                nc.scalar.copy(tip32_t[:], tip_t[:])
            sub_order = list(range(NSUB))
            if POOL_BC_F32:
                sub_order = [NSUB - 1] + sub_order[:-1]
            for s in sub_order:
                sl = slice(s * JP, (s + 1) * JP)
                pool_sub = POOL_BC_F32 and s == NSUB - 1
                sdst = tg32_t if pool_sub else tg_t
                sbase = 0 if pool_sub else s * 1536
                act(th_t[:, sl], la_t[:, sl], AFT.Tanh)
                act(la_t[:, sl], th_t[:, sl], AFT.Abs)
                # (per-axis z-first trig was tried to shave the head further:
                # +16 ACT op-inits cost more than the earlier availability
                # bought — 115.0us vs 113.0us. Keep whole-sub trig ops.)
                act(sdst[:, sbase:sbase + JP], th_t[:, sl], AFT.Sin, scale=sc)
                act(sdst[:, sbase + JP:sbase + 1536], la_t[:, sl],
                    AFT.Sin, bias=hpi_t[:], scale=-sc)

            # ---------------- BC: locals via Givens chain -----------------
            # dims (sub, l, e); sub range [s0, s0+ns) fused per op
            def emit_bc(eng_tt, s0, ns):
                tr4 = {}
                for nm, off in (("sx", 0), ("sy", 256), ("sz", 512),
                                ("cx", JP), ("cy", JP + 256),
                                ("cz", JP + 512)):
                    tr4[nm] = ap(tg_t, s0 * 1536 + off,
                                 [[1536, ns], [0, 4], [1, 256]])
                T = [ap(tip_t, k * 1024, [[0, ns], [256, 4], [1, 256]])
                     for k in range(3)]
                L = [ap(loc_t, s0 * OUTW + k * 1024,
                        [[OUTW, ns], [256, 4], [1, 256]]) for k in range(3)]
                R0 = ap(r0_t, s0 * 1024, [[1024, ns], [256, 4], [1, 256]])
                R1 = ap(r1_t, s0 * 1024, [[1024, ns], [256, 4], [1, 256]])
                Q2 = ap(q2_t, s0 * 1024, [[1024, ns], [256, 4], [1, 256]])
                TA = ap(tA_t, s0 * 1024, [[1024, ns], [256, 4], [1, 256]])
                triples = [
                    (tr4["cz"], T[0], tr4["sz"], T[1], R0, Alu.subtract, True),
                    (tr4["sz"], T[0], tr4["cz"], T[1], R1, Alu.add, False),
                    (tr4["cy"], R0, tr4["sy"], T[2], L[0], Alu.add, False),
                    (tr4["sy"], R0, tr4["cy"], T[2], Q2, Alu.subtract, False),
                    (tr4["cx"], R1, tr4["sx"], Q2, L[1], Alu.subtract, True),
                    (tr4["sx"], R1, tr4["cx"], Q2, L[2], Alu.add, False),
                ]
                for (a, b, c, d, dst, op, ta_first) in triples:
                    eng_tt(TA, a, b, Alu.mult)
                    eng_tt(dst, c, d, Alu.mult)
                    if ta_first:
                        eng_tt(dst, TA, dst, op)
                    else:
                        eng_tt(dst, dst, TA, op)

            def emit_bc32():
                # sub-3 BC on Pool, fp32 island
                ptt = nc.gpsimd.tensor_tensor
                tr4 = {}
                for nm, off in (("sx", 0), ("sy", 256), ("sz", 512),
                                ("cx", JP), ("cy", JP + 256),
                                ("cz", JP + 512)):
                    tr4[nm] = ap(tg32_t, off, [[0, 4], [1, 256]])
                T = [ap(tip32_t, k * 1024, [[256, 4], [1, 256]])
                     for k in range(3)]
                L = [ap(loc32_t, k * 1024, [[256, 4], [1, 256]])
                     for k in range(3)]
                R0 = ap(r032_t, 0, [[256, 4], [1, 256]])
                R1 = ap(r132_t, 0, [[256, 4], [1, 256]])
                Q2 = ap(q232_t, 0, [[256, 4], [1, 256]])
                TA = ap(tA32_t, 0, [[256, 4], [1, 256]])
                triples = [
                    (tr4["cz"], T[0], tr4["sz"], T[1], R0, Alu.subtract, True),
                    (tr4["sz"], T[0], tr4["cz"], T[1], R1, Alu.add, False),
                    (tr4["cy"], R0, tr4["sy"], T[2], L[0], Alu.add, False),
                    (tr4["sy"], R0, tr4["cy"], T[2], Q2, Alu.subtract, False),
                    (tr4["cx"], R1, tr4["sx"], Q2, L[1], Alu.subtract, True),
                    (tr4["sx"], R1, tr4["cx"], Q2, L[2], Alu.add, False),
                ]
                for (a, b, c, d, dst, op, ta_first) in triples:
                    ptt(TA, a, b, Alu.mult)
                    ptt(dst, c, d, Alu.mult)
                    if ta_first:
                        ptt(dst, TA, dst, op)
                    else:
                        ptt(dst, dst, TA, op)

            if POOL_BC_F32:
                emit_bc32()
                emit_bc(tt, 0, NSUB - 1)
                # cast the Pool island's locals to fp16, one op per k-plane
                # (fires as soon as that plane's chain completes)
                for k in range(3):
                    nc.scalar.copy(
                        loc_t[:, (NSUB - 1) * OUTW + k * 1024:
                              (NSUB - 1) * OUTW + (k + 1) * 1024],
                        loc32_t[:, k * 1024:(k + 1) * 1024])
            elif POOL_BC_SUB is None:
                # asymmetric groups {0} + {1,2,3}: DVE starts after just one
                # sub's trig (~3.3us); ACT finishes the rest under group-0's
                # ~10.7us of DVE work. Same 36 ops / same cycles as 2+2.
                emit_bc(tt, 0, 1)
                emit_bc(tt, 1, 3)
            elif POOL_BC_SUB == 0:
                emit_bc(nc.gpsimd.tensor_tensor, 0, 1)
                emit_bc(tt, 1, NSUB - 1)
            elif POOL_BC_SUB == NSUB - 1:
                emit_bc(nc.gpsimd.tensor_tensor, NSUB - 1, 1)
                emit_bc(tt, 0, NSUB - 1)
            else:
                raise ValueError(POOL_BC_SUB)

            # ---------------- pair product: loc[slot1] = T127 o T255 -------
            # (node 127 local at slot 128, node 255 local at slot 0);
            # sub-fused tiny ops, dims (sub, i, l)
            for k in range(3):
                a127 = ap(loc_t, k * 256 + 128, [[OUTW, 4], [1024, 3], [0, 4]])
                b255 = ap(loc_t, k * 1024, [[OUTW, 4], [0, 3], [256, 4]])
                if k == 0:
                    dst = ap(loc_t, 1, [[OUTW, 4], [1024, 3], [256, 4]])
                else:
                    dst = ap(tmp_t, (k - 1) * 1536,
                             [[OUTW, 4], [512, 3], [128, 4]])
                tt(dst, a127, b255, Alu.mult)
            d1 = ap(loc_t, 1, [[OUTW, 4], [1024, 3], [256, 4]])
            for k in (1, 2):
                tk = ap(tmp_t, (k - 1) * 1536, [[OUTW, 4], [512, 3], [128, 4]])
                tt(d1, d1, tk, Alu.add)
            tt(ap(loc_t, 768 + 1, [[OUTW, 4], [1024, 3], [1, 1]]),
               ap(loc_t, 768 + 1, [[OUTW, 4], [1024, 3], [1, 1]]),
               ap(loc_t, 768 + 128, [[OUTW, 4], [1024, 3], [1, 1]]), Alu.add)

            # ---------------- tree ----------------
            # root (slot 1) = identity (on Pool; frees DVE ops)
            nc.gpsimd.memset(ap(w_t, 1, [[OUTW, 4], [256, 12], [1, 1]]), 0.0)
            nc.gpsimd.memset(ap(w_t, 1, [[OUTW, 4], [1280, 3], [1, 1]]), 1.0)

            def child_ops(s, par_slot, chl_slot, loc_slot, q):
                """q children at w slots [chl_slot, chl_slot+q) from parents
                [par_slot, par_slot+q) and locals [loc_slot, loc_slot+q).
                (ISA mem patterns cap compute APs at 3 free dims, so subs
                cannot be fused here.)"""
                ws = s * OUTW
                for k in range(3):
                    par = ap(w_t, ws + k * 256 + par_slot,
                             [[1024, 3], [0, 4], [1, q]])
                    lsrc = ap(loc_t, ws + k * 1024 + loc_slot,
                              [[0, 3], [256, 4], [1, q]])
                    if k == 0:
                        dst = ap(w_t, ws + chl_slot,
                                 [[1024, 3], [256, 4], [1, q]])
                    else:
                        dst = ap(tmp_t, ws + (k - 1) * 1536 + (chl_slot % 128),
                                 [[512, 3], [128, 4], [1, q]])
                    tt(dst, par, lsrc, Alu.mult)

            # level 2: W = I @ local -> plain copy of loc slots [2,4) (on the
            # otherwise-idle ACT engine)
            nc.scalar.copy(
                ap(w_t, 2, [[OUTW, 4], [256, 12], [1, 2]]),
                ap(loc_t, 2, [[OUTW, 4], [256, 12], [1, 2]]))

            for lvl in range(3, 9):
                blk = 1 << (lvl - 1)
                q = blk >> 1
                pb = q
                if lvl == 8:
                    # per-side so each half's output DMA starts as soon as
                    # that half is done (shrinks the post-compute DMA tail)
                    for side in (0, 1):
                        c = blk + side * q
                        for s in range(NSUB):
                            child_ops(s, pb, c, c, q)
                        # last side: adds/tr/DMA per 32-slot half so the
                        # first half's DMA hides under the second's adds
                        chunks = ((0, q),) if side == 0 else ((0, 32), (32, 32))
                        for (h0, hq) in chunks:
                            ch = c + h0
                            for k in (1, 2):
                                wd = ap(w_t, ch,
                                        [[OUTW, 4], [256, 12], [1, hq]])
                                td = ap(tmp_t, (k - 1) * 1536 + (ch % 128),
                                        [[OUTW, 4], [128, 12], [1, hq]])
                                tt(wd, wd, td, Alu.add)
                            wd = ap(w_t, 768 + ch,
                                    [[OUTW, 4], [1024, 3], [1, hq]])
                            ps = ap(w_t, 768 + pb + h0,
                                    [[OUTW, 4], [1024, 3], [1, hq]])
                            tt(wd, wd, ps, Alu.add)
                            for s in range(NSUB):
                                nc.sync.dma_start(
                                    bass.AP(out_d, s * P * OUTW + ch,
                                            [[OUTW, P], [256, 12], [1, hq]]),
                                    ap(w_t, s * OUTW + ch,
                                       [[256, 12], [1, hq]]))
                    continue
                for s in range(NSUB):
                    for side in (0, 1):
                        child_ops(s, pb, blk + side * q, blk + side * q, q)
                for k in (1, 2):
                    wd = ap(w_t, blk, [[OUTW, 4], [256, 12], [1, 2 * q]])
                    td = ap(tmp_t, (k - 1) * 1536 + (blk % 128),
                            [[OUTW, 4], [128, 12], [1, 2 * q]])
                    tt(wd, wd, td, Alu.add)
                for side in (0, 1):
                    wd = ap(w_t, 768 + blk + side * q,
                            [[OUTW, 4], [1024, 3], [1, q]])
                    ps = ap(w_t, 768 + pb, [[OUTW, 4], [1024, 3], [1, q]])
                    tt(wd, wd, ps, Alu.add)

                if lvl == 7:
                    # node 255 -> slot 0 (parent node 63 at slot 64, local =
                    # pair product at loc slot 1); sub-fused tiny ops
                    for k in range(3):
                        par = ap(w_t, k * 256 + 64,
                                 [[OUTW, 4], [1024, 3], [0, 4]])
                        lsrc = ap(loc_t, k * 1024 + 1,
                                  [[OUTW, 4], [0, 3], [256, 4]])
                        if k == 0:
                            dst = ap(w_t, 0, [[OUTW, 4], [1024, 3], [256, 4]])
                        else:
                            dst = ap(tmp_t, (k - 1) * 1536 + 64,
                                     [[OUTW, 4], [512, 3], [128, 4]])
                        tt(dst, par, lsrc, Alu.mult)
                    wd = ap(w_t, 0, [[OUTW, 4], [1024, 3], [256, 4]])
                    for k in (1, 2):
                        tk = ap(tmp_t, (k - 1) * 1536 + 64,
                                [[OUTW, 4], [512, 3], [128, 4]])
                        tt(wd, wd, tk, Alu.add)
                    tt(ap(w_t, 768 + 0, [[OUTW, 4], [1024, 3], [1, 1]]),
                       ap(w_t, 768 + 0, [[OUTW, 4], [1024, 3], [1, 1]]),
                       ap(w_t, 768 + 64, [[OUTW, 4], [1024, 3], [1, 1]]),
                       Alu.add)
                    # output DMAs for everything below level 8
                    for s in range(NSUB):
                        nc.sync.dma_start(
                            bass.AP(out_d, s * P * OUTW,
                                    [[OUTW, P], [256, 12], [1, 64]]),
                            ap(w_t, s * OUTW, [[256, 12], [1, 64]]))
                        nc.sync.dma_start(
                            bass.AP(out_d, s * P * OUTW + 64,
                                    [[OUTW, P], [256, 12], [1, 64]]),
                            ap(w_t, s * OUTW + 64, [[256, 12], [1, 64]]))

    nc.compile()
    return nc


# --------------------------------------------------------------------------- #
# cached PJRT runner (axon path) — compile once, execute per call
# --------------------------------------------------------------------------- #
def _get_runner(general_constraints, sc_const, of_const, loop_n=1):
    key = ("runner", round(float(sc_const), 6), loop_n)
    if key in _state:
        return _state[key]

    import jax
    from jax.sharding import Mesh, PartitionSpec, NamedSharding
    from jax.experimental.shard_map import shard_map
    import concourse.mybir as mybir
    from concourse import bass2jax

    nc = _build_nc(float(sc_const), loop_n)
    bass2jax.install_neuronx_cc_hook()

    part_name = (nc.partition_id_tensor.name
                 if nc.partition_id_tensor is not None else None)
    in_names, out_names, out_avals = [], [], []
    for alloc in nc.m.functions[0].allocations:
        if not isinstance(alloc, mybir.MemoryLocationSet):
            continue
        name = alloc.memorylocations[0].name
        if alloc.kind == "ExternalInput":
            if name != part_name:
                in_names.append(name)
        elif alloc.kind == "ExternalOutput":
            out_names.append(name)
            out_avals.append(jax.core.ShapedArray(
                tuple(alloc.tensor_shape), mybir.dt.np(alloc.dtype)))
    n_params = len(in_names)
    all_in = in_names + out_names
    if part_name is not None:
        all_in = all_in + [part_name]

    def _body(*args):
        operands = list(args)
        if part_name is not None:
            operands.append(bass2jax.partition_id_tensor())
        outs = bass2jax._bass_exec_p.bind(
            *operands,
            out_avals=tuple(out_avals),
            in_names=tuple(all_in),
            out_names=tuple(out_names),
            lowering_input_output_aliases=(),
            sim_require_finite=True,
            sim_require_nnan=True,
            nc=nc,
        )
        return tuple(outs)

    devices = jax.devices()[:NCORE]
    mesh = Mesh(np.asarray(devices), ("core",))
    nin = n_params + len(out_names)
    sharded = jax.jit(
        shard_map(_body, mesh=mesh,
                  in_specs=(PartitionSpec("core"),) * nin,
                  out_specs=(PartitionSpec("core"),) * len(out_names),
                  check_rep=False),
        donate_argnums=tuple(range(n_params, nin)),
        keep_unused=True,
    )
    shard0 = NamedSharding(mesh, PartitionSpec("core"))

    def _make_zeros():
        return jax.jit(
            lambda: jax.numpy.zeros((NCORE * BPC, OUTW), np.float16),
            out_shardings=shard0)()

    runner = (sharded, in_names, _make_zeros)
    _state[key] = runner
    return runner


def _run_device(la_feed, tip_feed, sc_const):
    sharded, in_names, make_zeros = _get_runner(False, sc_const, 0.0)
    feed = {
        "la": np.ascontiguousarray(la_feed, dtype=np.float32),
        "tip": np.broadcast_to(tip_feed.reshape(1, OUTW),
                               (NCORE, OUTW)).copy(),
    }
    args = [feed[name] for name in in_names]
    out = sharded(*args, make_zeros())[0]
    return assemble_output(out)


# --------------------------------------------------------------------------- #
# public entry point
# --------------------------------------------------------------------------- #
def kernel(log_angles, tip_to_base, rot_axes, rot_constraints):
    log_angles = np.asarray(log_angles)
    tip_to_base = np.asarray(tip_to_base)
    rot_axes = np.asarray(rot_axes)
    rot_constraints = np.asarray(rot_constraints)

    expected_shapes = (log_angles.shape == (B, J)
                       and tip_to_base.shape == (E, 4, 4)
                       and rot_axes.shape == (J, 3)
                       and rot_constraints.shape == (J, 2))
    eye_tiled = np.tile(np.eye(3, dtype=np.float32), (E, 1)) \
        if expected_shapes else None
    euler = expected_shapes and np.allclose(rot_axes, eye_tiled, atol=1e-6)
    if not euler:
        return _np_fallback(log_angles, tip_to_base, rot_axes, rot_constraints)

    sc = rot_constraints[:, 0].astype(np.float32)
    of = rot_constraints[:, 1].astype(np.float32)
    const_ok = (np.all(sc == sc[0]) and np.all(of == 0.0)
                and float(sc[0]) > 1e-3
                and float(sc[0]) <= PI + 1e-4)
    if not const_ok:
        return _np_fallback(log_angles, tip_to_base, rot_axes,
                            rot_constraints)

    la_feed, tip_feed = make_feeds(log_angles, tip_to_base)
    return _run_device(la_feed, tip_feed, float(sc[0]))
